# revision 1
# baseline (speedup 1.0000x reference)
"""Trainium2 Bass kernel for nn_DeformableStripAttention_68461778698537.

Sharding: 8 cores = (b, h) pairs (B=2 x HEADS=4), per sharding hint
("shard the heads axis ... each head's grid_sample+attention is independent").
Each core computes its head's Q/K/V projections (the dominant dense matmuls)
from the full per-sample input via TensorE. The data-dependent deformable
gather + per-pixel attention tail is finished on host (numpy), exactly
mirroring the reference math in f32.
"""
import sys
sys.path.insert(0, "/opt/trn_rl_repo")
import numpy as np

DIM = 256
HEADS = 4
STRIPS = 4
M = 8
MAX_OFF = 0.5
B = 2
H = 64
W = 64
HD = DIM // HEADS
P = H * W
SCALE = HD ** -0.5
GN_EPS = 1e-5
N_CORES = 8

_CACHE = {}


def _build_nc():
    import concourse.bacc as bacc
    import concourse.mybir as mybir
    import concourse.tile as tile

    f32 = mybir.dt.float32
    nc = bacc.Bacc("TRN2", target_bir_lowering=False, debug=False,
                   num_devices=N_CORES)
    xb = nc.dram_tensor("xb", [DIM, P], f32, kind="ExternalInput")
    wqt = nc.dram_tensor("wqt", [DIM, HD], f32, kind="ExternalInput")
    wkt = nc.dram_tensor("wkt", [DIM, HD], f32, kind="ExternalInput")
    wvt = nc.dram_tensor("wvt", [DIM, HD], f32, kind="ExternalInput")
    oq = nc.dram_tensor("oq", [HD, P], f32, kind="ExternalOutput")
    ok = nc.dram_tensor("ok", [HD, P], f32, kind="ExternalOutput")
    ov = nc.dram_tensor("ov", [HD, P], f32, kind="ExternalOutput")

    with tile.TileContext(nc) as tc:
        with tc.tile_pool(name="sbuf", bufs=2) as pool, \
             tc.tile_pool(name="psum", bufs=2, space="PSUM") as psum:
            xt = [pool.tile([128, P], f32, tag="x", name=f"xt{i}")
                  for i in range(2)]
            for c in range(2):
                nc.sync.dma_start(xt[c][:], xb[128 * c:128 * (c + 1), :])
            for wt, outt in ((wqt, oq), (wkt, ok), (wvt, ov)):
                wtl = [pool.tile([128, HD], f32, tag="w", name=f"w{id(wt)}{i}")
                       for i in range(2)]
                for c in range(2):
                    nc.sync.dma_start(wtl[c][:], wt[128 * c:128 * (c + 1), :])
                res = pool.tile([HD, P], f32, tag="res", name=f"res{id(wt)}")
                for j in range(P // 512):
                    acc = psum.tile([HD, 512], f32, tag="acc",
                                    name=f"acc{id(wt)}_{j}")
                    for c in range(2):
                        nc.tensor.matmul(acc[:], wtl[c][:],
                                         xt[c][:, 512 * j:512 * (j + 1)],
                                         start=(c == 0), stop=(c == 1))
                    nc.vector.tensor_copy(res[:, 512 * j:512 * (j + 1)], acc[:])
                nc.sync.dma_start(outt[:], res[:])
    nc.compile()
    return nc


def _host_tail(q, k, v, x, Wo, bo, dir_W, dir_b, gn_w, gn_b):
    """q/k/v: [B, HEADS, HD, P] f32. Mirrors reference math in numpy f32."""
    Bx = B
    avg = x.reshape(Bx, DIM, P).mean(axis=2)                      # [B, C]
    dirs = (avg @ dir_W.T + dir_b).reshape(Bx, HEADS, STRIPS, 2)
    nrm = np.maximum(np.linalg.norm(dirs, axis=-1, keepdims=True), 1e-6)
    dirs = dirs / nrm

    t = np.linspace(-MAX_OFF, MAX_OFF, M, dtype=np.float32)
    off = dirs[:, :, :, None, :] * t[None, None, None, :, None]   # [B,h,S,M,2]
    ys = np.linspace(-1.0, 1.0, H, dtype=np.float32)
    xs = np.linspace(-1.0, 1.0, W, dtype=np.float32)
    gy, gx = np.meshgrid(ys, xs, indexing="ij")
    base = np.stack([gx, gy], axis=-1).reshape(P, 2).astype(np.float32)
    g = np.clip(base[None, None, None, :, None, :]
                + off[:, :, :, None, :, :], -1.0, 1.0)            # [B,h,S,P,M,2]
    px = (g[..., 0] + 1.0) * 0.5 * (W - 1)
    py = (g[..., 1] + 1.0) * 0.5 * (H - 1)
    x0 = np.floor(px)
    y0 = np.floor(py)
    fx = (px - x0)[:, :, None]                                    # [B,h,1,S,P,M]
    fy = (py - y0)[:, :, None]
    x0i = np.clip(x0.astype(np.int32), 0, W - 1)
    x1i = np.clip(x0i + 1, 0, W - 1)
    y0i = np.clip(y0.astype(np.int32), 0, H - 1)
    y1i = np.clip(y0i + 1, 0, H - 1)

    def gather(feat, yi, xi):
        # feat [B,h,hd,P]; yi,xi [B,h,S,P,M] -> [B,h,hd,S,P,M]
        idx = (yi * W + xi).reshape(Bx, HEADS, 1, -1)
        idx = np.broadcast_to(idx, (Bx, HEADS, HD, idx.shape[-1]))
        out = np.take_along_axis(feat, idx, axis=-1)
        return out.reshape(Bx, HEADS, HD, STRIPS, P, M)

    def bilinear(feat):
        v00 = gather(feat, y0i, x0i)
        v01 = gather(feat, y0i, x1i)
        v10 = gather(feat, y1i, x0i)
        v11 = gather(feat, y1i, x1i)
        return (v00 * (1 - fx) * (1 - fy) + v01 * fx * (1 - fy)
                + v10 * (1 - fx) * fy + v11 * fx * fy)

    kf = k.reshape(Bx, HEADS, HD, P)
    vf = v.reshape(Bx, HEADS, HD, P)
    sK = bilinear(kf).transpose(0, 1, 4, 3, 5, 2).reshape(
        Bx, HEADS, P, STRIPS * M, HD)
    sV = bilinear(vf).transpose(0, 1, 4, 3, 5, 2).reshape(
        Bx, HEADS, P, STRIPS * M, HD)

    qf = q.reshape(Bx, HEADS, HD, P).transpose(0, 1, 3, 2)        # [B,h,P,hd]
    attn = np.einsum("bhpd,bhpkd->bhpk", qf, sK) * SCALE
    attn = attn - attn.max(axis=-1, keepdims=True)
    attn = np.exp(attn)
    attn = attn / attn.sum(axis=-1, keepdims=True)
    o = np.einsum("bhpk,bhpkd->bhpd", attn, sV)                   # [B,h,P,hd]
    out = o.transpose(0, 1, 3, 2).reshape(Bx, DIM, H, W)

    grp = out.reshape(Bx, HEADS, -1)
    mu = grp.mean(-1, keepdims=True)
    var = grp.var(-1, keepdims=True)
    gn = ((grp - mu) / np.sqrt(var + GN_EPS)).reshape(Bx, DIM, H, W)
    gn = gn * gn_w[None, :, None, None] + gn_b[None, :, None, None]

    out = np.einsum("oc,bchw->bohw", Wo, gn) + bo[None, :, None, None]
    return (out + x).astype(np.float32)


def kernel(**inputs):
    from concourse.bass_utils import run_bass_kernel_spmd

    x = np.asarray(inputs["x"], dtype=np.float32)
    Wq = np.asarray(inputs["Wq"], dtype=np.float32)
    Wk = np.asarray(inputs["Wk"], dtype=np.float32)
    Wv = np.asarray(inputs["Wv"], dtype=np.float32)
    Wo = np.asarray(inputs["Wo"], dtype=np.float32)
    bo = np.asarray(inputs["bo"], dtype=np.float32)
    dir_W = np.asarray(inputs["dir_W"], dtype=np.float32)
    dir_b = np.asarray(inputs["dir_b"], dtype=np.float32)
    gn_w = np.asarray(inputs["gn_w"], dtype=np.float32)
    gn_b = np.asarray(inputs["gn_b"], dtype=np.float32)

    if "nc" not in _CACHE:
        _CACHE["nc"] = _build_nc()
    nc = _CACHE["nc"]

    in_maps = []
    for core in range(N_CORES):
        b, h = core // HEADS, core % HEADS
        sl = slice(HD * h, HD * (h + 1))
        in_maps.append({
            "xb": np.ascontiguousarray(x[b].reshape(DIM, P)),
            "wqt": np.ascontiguousarray(Wq[sl, :].T),
            "wkt": np.ascontiguousarray(Wk[sl, :].T),
            "wvt": np.ascontiguousarray(Wv[sl, :].T),
        })
    res = run_bass_kernel_spmd(nc, in_maps, core_ids=list(range(N_CORES)))

    q = np.zeros((B, HEADS, HD, P), np.float32)
    k = np.zeros((B, HEADS, HD, P), np.float32)
    v = np.zeros((B, HEADS, HD, P), np.float32)
    for core in range(N_CORES):
        b, h = core // HEADS, core % HEADS
        q[b, h] = res.results[core]["oq"]
        k[b, h] = res.results[core]["ok"]
        v[b, h] = res.results[core]["ov"]

    return _host_tail(q, k, v, x, Wo, bo, dir_W, dir_b, gn_w, gn_b)



# revision 2
# speedup vs baseline: 31.1648x; 31.1648x over previous
"""Trainium2 Bass kernel for nn_DeformableStripAttention_68461778698537.

Sharding: 8 cores = (b, h) pairs (B=2 x HEADS=4), per sharding hint
("shard the heads axis ... each head's grid_sample+attention is independent").
Each core computes its head's Q/K/V projections (the dominant dense matmuls)
from the full per-sample input via TensorE. The data-dependent deformable
gather + per-pixel attention tail is finished on host (numpy), exactly
mirroring the reference math in f32.
"""
import sys
sys.path.insert(0, "/opt/trn_rl_repo")
import numpy as np

DIM = 256
HEADS = 4
STRIPS = 4
M = 8
MAX_OFF = 0.5
B = 2
H = 64
W = 64
HD = DIM // HEADS
P = H * W
SCALE = HD ** -0.5
GN_EPS = 1e-5
N_CORES = 8

_CACHE = {}


def _build_nc():
    import concourse.bacc as bacc
    import concourse.mybir as mybir
    import concourse.tile as tile

    f32 = mybir.dt.float32
    nc = bacc.Bacc("TRN2", target_bir_lowering=False, debug=False,
                   num_devices=N_CORES)
    xb = nc.dram_tensor("xb", [DIM, P], f32, kind="ExternalInput")
    wqt = nc.dram_tensor("wqt", [DIM, HD], f32, kind="ExternalInput")
    wkt = nc.dram_tensor("wkt", [DIM, HD], f32, kind="ExternalInput")
    wvt = nc.dram_tensor("wvt", [DIM, HD], f32, kind="ExternalInput")
    oq = nc.dram_tensor("oq", [HD, P], f32, kind="ExternalOutput")
    ok = nc.dram_tensor("ok", [HD, P], f32, kind="ExternalOutput")
    ov = nc.dram_tensor("ov", [HD, P], f32, kind="ExternalOutput")

    with tile.TileContext(nc) as tc:
        with tc.tile_pool(name="sbuf", bufs=2) as pool, \
             tc.tile_pool(name="psum", bufs=2, space="PSUM") as psum:
            xt = [pool.tile([128, P], f32, tag="x", name=f"xt{i}")
                  for i in range(2)]
            for c in range(2):
                nc.sync.dma_start(xt[c][:], xb[128 * c:128 * (c + 1), :])
            for wt, outt in ((wqt, oq), (wkt, ok), (wvt, ov)):
                wtl = [pool.tile([128, HD], f32, tag="w", name=f"w{id(wt)}{i}")
                       for i in range(2)]
                for c in range(2):
                    nc.sync.dma_start(wtl[c][:], wt[128 * c:128 * (c + 1), :])
                res = pool.tile([HD, P], f32, tag="res", name=f"res{id(wt)}")
                for j in range(P // 512):
                    acc = psum.tile([HD, 512], f32, tag="acc",
                                    name=f"acc{id(wt)}_{j}")
                    for c in range(2):
                        nc.tensor.matmul(acc[:], wtl[c][:],
                                         xt[c][:, 512 * j:512 * (j + 1)],
                                         start=(c == 0), stop=(c == 1))
                    nc.vector.tensor_copy(res[:, 512 * j:512 * (j + 1)], acc[:])
                nc.sync.dma_start(outt[:], res[:])
    nc.compile()
    return nc


def _shift_params(x, dir_W, dir_b):
    """Per-(b,h,s,m) integer shift + frac weights for the strip sampling.

    The sampling offset is constant across pixels, so grid_sample reduces to a
    constant fractional shift of the whole image with replicate border:
    clip+bilinear == 4 integer-shifted edge-clamped copies with fixed weights.
    """
    avg = x.reshape(B, DIM, P).mean(axis=2)
    dirs = (avg @ dir_W.T + dir_b).reshape(B, HEADS, STRIPS, 2)
    dirs = dirs / np.maximum(np.linalg.norm(dirs, axis=-1, keepdims=True), 1e-6)
    t = np.linspace(-MAX_OFF, MAX_OFF, M)
    dx = dirs[..., 0:1] * t * 0.5 * (W - 1)                       # [B,h,S,M]
    dy = dirs[..., 1:2] * t * 0.5 * (H - 1)
    dxi = np.floor(dx).astype(np.int64)
    dyi = np.floor(dy).astype(np.int64)
    fx = (dx - dxi).astype(np.float32)
    fy = (dy - dyi).astype(np.float32)
    return dxi, dyi, fx, fy


def _host_tail(q, k, v, x, Wo, bo, dir_W, dir_b, gn_w, gn_b):
    """q/k/v: [B, HEADS, HD, P] f32. Shift-trick tail, numpy f32."""
    dxi, dyi, fx, fy = _shift_params(x, dir_W, dir_b)
    SM = STRIPS * M
    out = np.empty((B, DIM, P), np.float32)
    for b in range(B):
        xb = x[b].reshape(DIM, P)
        acc = np.zeros((DIM, P), np.float32)
        for h in range(HEADS):
            Kp = np.pad(k[b, h].reshape(HD, H, W), ((0, 0), (16, 16), (16, 16)),
                        mode="edge")
            Vp = np.pad(v[b, h].reshape(HD, H, W), ((0, 0), (16, 16), (16, 16)),
                        mode="edge")
            sK = np.empty((SM, HD, P), np.float32)
            sV = np.empty((SM, HD, P), np.float32)
            for s in range(STRIPS):
                for m in range(M):
                    yo = 16 + dyi[b, h, s, m]
                    xo = 16 + dxi[b, h, s, m]
                    wx1, wy1 = fx[b, h, s, m], fy[b, h, s, m]
                    sm = s * M + m
                    first = True
                    for cy, wyc in ((0, 1.0 - wy1), (1, wy1)):
                        for cx, wxc in ((0, 1.0 - wx1), (1, wx1)):
                            wgt = np.float32(wyc * wxc)
                            slk = Kp[:, yo + cy:yo + cy + H,
                                     xo + cx:xo + cx + W].reshape(HD, P)
                            slv = Vp[:, yo + cy:yo + cy + H,
                                     xo + cx:xo + cx + W].reshape(HD, P)
                            if first:
                                np.multiply(slk, wgt, out=sK[sm])
                                np.multiply(slv, wgt, out=sV[sm])
                                first = False
                            else:
                                sK[sm] += wgt * slk
                                sV[sm] += wgt * slv
            scores = np.einsum("cp,kcp->kp", q[b, h], sK,
                               optimize=True) * SCALE              # [SM,P]
            scores -= scores.max(axis=0, keepdims=True)
            np.exp(scores, out=scores)
            scores /= scores.sum(axis=0, keepdims=True)
            o = np.einsum("kp,kcp->cp", scores, sV, optimize=True)  # [HD,P]
            mu = o.mean()
            inv = 1.0 / np.sqrt(o.var() + GN_EPS)
            sl = slice(h * HD, (h + 1) * HD)
            gn = (o - mu) * inv * gn_w[sl, None] + gn_b[sl, None]
            acc += Wo[:, sl] @ gn
        out[b] = acc + bo[:, None] + xb
    return out.reshape(B, DIM, H, W)


def kernel(**inputs):
    from concourse.bass_utils import run_bass_kernel_spmd

    x = np.asarray(inputs["x"], dtype=np.float32)
    Wq = np.asarray(inputs["Wq"], dtype=np.float32)
    Wk = np.asarray(inputs["Wk"], dtype=np.float32)
    Wv = np.asarray(inputs["Wv"], dtype=np.float32)
    Wo = np.asarray(inputs["Wo"], dtype=np.float32)
    bo = np.asarray(inputs["bo"], dtype=np.float32)
    dir_W = np.asarray(inputs["dir_W"], dtype=np.float32)
    dir_b = np.asarray(inputs["dir_b"], dtype=np.float32)
    gn_w = np.asarray(inputs["gn_w"], dtype=np.float32)
    gn_b = np.asarray(inputs["gn_b"], dtype=np.float32)

    if "nc" not in _CACHE:
        _CACHE["nc"] = _build_nc()
    nc = _CACHE["nc"]

    in_maps = []
    for core in range(N_CORES):
        b, h = core // HEADS, core % HEADS
        sl = slice(HD * h, HD * (h + 1))
        in_maps.append({
            "xb": np.ascontiguousarray(x[b].reshape(DIM, P)),
            "wqt": np.ascontiguousarray(Wq[sl, :].T),
            "wkt": np.ascontiguousarray(Wk[sl, :].T),
            "wvt": np.ascontiguousarray(Wv[sl, :].T),
        })
    res = run_bass_kernel_spmd(nc, in_maps, core_ids=list(range(N_CORES)))

    q = np.zeros((B, HEADS, HD, P), np.float32)
    k = np.zeros((B, HEADS, HD, P), np.float32)
    v = np.zeros((B, HEADS, HD, P), np.float32)
    for core in range(N_CORES):
        b, h = core // HEADS, core % HEADS
        q[b, h] = res.results[core]["oq"]
        k[b, h] = res.results[core]["ok"]
        v[b, h] = res.results[core]["ov"]

    return _host_tail(q, k, v, x, Wo, bo, dir_W, dir_b, gn_w, gn_b)



# revision 3
# speedup vs baseline: 75.8367x; 2.4334x over previous
"""Full on-device Trainium2 kernel for nn_DeformableStripAttention.

Sharding: 8 cores = (b, h) pairs. Each core runs the whole per-head pipeline:
QKV projection, deformable strip sampling, attention, GroupNorm, and its
partial of the output projection. Host computes only the tiny shift tables
(from the direction head) and sums the 4 per-head partials per sample.

Key math: the sampling offset is constant across pixels, so grid_sample
reduces to a constant fractional shift of the whole image with replicate
border == 4 integer-shifted edge-clamped copies blended with fixed weights.
On device that is 4 fused multiply-adds reading a 96x96 replicate-padded
image at dynamic offsets (per strip-sample), with K and V packed on the
128 partitions.
"""
import sys
sys.path.insert(0, "/opt/trn_rl_repo")
import numpy as np
from ml_dtypes import bfloat16

DIM = 256
HEADS = 4
STRIPS = 4
M = 8
SM = STRIPS * M
MAX_OFF = 0.5
B = 2
H = 64
W = 64
HD = DIM // HEADS
P = H * W
PADW = 96
SCALE = HD ** -0.5
GN_EPS = 1e-5
N_CORES = 8

_CACHE = {}


def _build_nc(mode="full"):
    import concourse.bacc as bacc
    import concourse.mybir as mybir
    import concourse.tile as tile
    import concourse.bass as bass
    from concourse import masks

    f32 = mybir.dt.float32
    bf16 = mybir.dt.bfloat16
    i32 = mybir.dt.int32
    AF = mybir.ActivationFunctionType
    OP = mybir.AluOpType
    AX = mybir.AxisListType
    ds = bass.ds

    nc = bacc.Bacc("TRN2", target_bir_lowering=False, debug=False,
                   num_devices=N_CORES)

    xb = nc.dram_tensor("xb", [DIM, P], bf16, kind="ExternalInput")
    wq = nc.dram_tensor("wq", [DIM, HD], bf16, kind="ExternalInput")
    wk = nc.dram_tensor("wk", [DIM, HD], bf16, kind="ExternalInput")
    wv = nc.dram_tensor("wv", [DIM, HD], bf16, kind="ExternalInput")
    wo = nc.dram_tensor("wo", [HD, DIM], f32, kind="ExternalInput")
    yoxo = nc.dram_tensor("yoxo", [1, 4 * SM], i32, kind="ExternalInput")
    wts = nc.dram_tensor("wts", [1, 4 * SM], f32, kind="ExternalInput")
    gnsb = nc.dram_tensor("gnsb", [HD, 2], f32, kind="ExternalInput")
    po = nc.dram_tensor("po", [DIM, P], bf16, kind="ExternalOutput")

    with tile.TileContext(nc) as tc:
        with tc.tile_pool(name="persist", bufs=1) as pp, \
             tc.tile_pool(name="work", bufs=2) as wp, \
             tc.tile_pool(name="small", bufs=3) as sp, \
             tc.tile_pool(name="psum", bufs=5, space="PSUM") as psp:

            # ---------------- constants & inputs to SBUF ----------------
            ident = pp.tile([128, 128], f32, tag="ident")
            masks.make_identity(nc, ident[:])
            identb = pp.tile([128, 128], bf16, tag="identb")
            masks.make_identity(nc, identb[:])
            ones_col = pp.tile([HD, 1], f32, tag="onesc")
            nc.gpsimd.memset(ones_col[:], 1.0)
            ones_row = pp.tile([1, HD], f32, tag="onesr")
            nc.gpsimd.memset(ones_row[:], 1.0)

            xt = [pp.tile([128, P], bf16, tag=f"xt{i}", name=f"xt{i}")
                  for i in range(2)]
            for c in range(2):
                nc.sync.dma_start(xt[c][:], xb[128 * c:128 * (c + 1), :])
            wq_t = pp.tile([128, 2 * HD], bf16, tag="wq")
            wk_t = pp.tile([128, 2 * HD], bf16, tag="wk")
            wv_t = pp.tile([128, 2 * HD], bf16, tag="wv")
            wo_t = pp.tile([HD, DIM], f32, tag="wo")
            for t_, d_ in ((wq_t, wq), (wk_t, wk), (wv_t, wv)):
                for c in range(2):
                    nc.sync.dma_start(t_[:, HD * c:HD * (c + 1)],
                                      d_[128 * c:128 * (c + 1), :])
            nc.sync.dma_start(wo_t[:], wo[:])
            yoxo_t = pp.tile([1, 4 * SM], i32, tag="yoxo")
            nc.sync.dma_start(yoxo_t[:], yoxo[:])
            wts_row = pp.tile([1, 4 * SM], f32, tag="wtsr")
            nc.sync.dma_start(wts_row[:], wts[:])
            ones_r128 = pp.tile([1, 128], f32, tag="onesr128")
            nc.gpsimd.memset(ones_r128[:], 1.0)
            wts_t = pp.tile([128, 4 * SM], f32, tag="wts")
            psw = psp.tile([128, 4 * SM], f32, tag="ps")
            nc.tensor.matmul(psw[:], ones_r128[:], wts_row[:],
                             start=True, stop=True)
            nc.scalar.copy(wts_t[:], psw[:])
            gnsb_t = pp.tile([HD, 2], f32, tag="gnsb")
            nc.sync.dma_start(gnsb_t[:], gnsb[:])

            # ---------------- QKV projections ----------------
            # K/V go straight into the interior of the padded KV image.
            kvpad = pp.tile([128, PADW * PADW], bf16, tag="kvpad")
            kvp3 = kvpad[:].rearrange("p (r c) -> p r c", c=PADW)
            qsb = pp.tile([HD, P], bf16, tag="qsb")

            for wt, kind in ((wq_t, "q"), (wk_t, "k"), (wv_t, "v")):
                for j in range(8):
                    ps = psp.tile([HD, 512], f32, tag="ps")
                    for c in range(2):
                        nc.tensor.matmul(
                            ps[:], wt[:, HD * c:HD * (c + 1)],
                            xt[c][:, 512 * j:512 * (j + 1)],
                            start=(c == 0), stop=(c == 1))
                    if kind == "q":
                        nc.scalar.copy(qsb[:, 512 * j:512 * (j + 1)], ps[:])
                    else:
                        prow = 0 if kind == "k" else HD
                        dst = kvp3[prow:prow + HD,
                                   16 + 8 * j:16 + 8 * (j + 1), 16:16 + W]
                        nc.scalar.copy(dst, ps[:].rearrange(
                            "p (r c) -> p r c", c=W))

            # ---------------- replicate-pad borders ----------------
            nc.vector.tensor_copy(
                kvp3[:, 16:16 + H, 0:16],
                kvp3[:, 16:16 + H, 16:17].to_broadcast([128, H, 16]))
            nc.vector.tensor_copy(
                kvp3[:, 16:16 + H, 16 + W:PADW],
                kvp3[:, 16:16 + H, 15 + W:16 + W].to_broadcast([128, H, 16]))
            nc.vector.tensor_copy(
                kvp3[:, 0:16, :],
                kvp3[:, 16:17, :].to_broadcast([128, 16, PADW]))
            nc.vector.tensor_copy(
                kvp3[:, 16 + H:PADW, :],
                kvp3[:, 15 + H:16 + H, :].to_broadcast([128, 16, PADW]))

            # ---------------- Q transposed to pixel-major ----------------
            qt = pp.tile([128, 2048], bf16, tag="qt")      # [p, t*64+ch]
            for t in range(32):
                ps = psp.tile([128, HD], bf16, tag="ps")
                nc.tensor.transpose(ps[:], qsb[:, 128 * t:128 * (t + 1)],
                                    identb[0:HD, 0:HD])
                nc.scalar.copy(qt[:, HD * t:HD * (t + 1)], ps[:])

            # ---------------- main sampling + attention loop ----------------
            acc = pp.tile([128, 2048], f32, tag="acc")     # [p, t*64+ch]
            probs = pp.tile([128, SM * 32], f32, tag="probs")  # col = t*SM+sm

            # Two y- and two x-corner registers, reloaded each sm. No symbolic
            # arithmetic on them (host sends all four coords) so no stale
            # value-cache entries can survive a reload.
            cregs = [nc.vector.alloc_register(f"creg{i}") for i in range(4)]
            cvals = [bass.RuntimeValue(r, min_val=0, max_val=32) for r in cregs]

            sm_range = range(0) if mode == "skeleton" else range(SM)
            for sm in sm_range:
                if mode == "full":
                    for i in range(4):
                        nc.vector.reg_load(
                            cregs[i], yoxo_t[0:1, SM * i + sm:SM * i + sm + 1])
                skv = wp.tile([128, P], bf16, tag="skv")
                skv3 = skv[:].rearrange("p (a b) -> p a b", b=W)
                for ci, (cy, cx) in enumerate(((0, 0), (0, 1), (1, 0), (1, 1))):
                    if mode == "full":
                        src = kvp3[:, ds(cvals[cy], H), ds(cvals[2 + cx], W)]
                    else:
                        src = kvp3[:, 16 + cy:16 + cy + H, 16 + cx:16 + cx + W]
                    wcol = wts_t[:, 4 * sm + ci:4 * sm + ci + 1]
                    if ci == 0:
                        nc.vector.tensor_scalar_mul(skv3, src, wcol)
                    else:
                        nc.vector.scalar_tensor_tensor(
                            skv3, src, wcol, skv3, OP.mult, OP.add)

                for g in range(8):
                    ps = psp.tile([128, 512], bf16, tag="ps")
                    for i in range(4):
                        t = 4 * g + i
                        nc.tensor.transpose(
                            ps[:, 128 * i:128 * (i + 1)],
                            skv[:, 128 * t:128 * (t + 1)], identb[:])
                    scg = sp.tile([128, 4], f32, tag="scg")
                    for i in range(4):
                        t = 4 * g + i
                        scr = sp.tile([128, HD], bf16, tag="ttr_scr")
                        # dot(qt_t, sampled K) per pixel: elementwise product
                        # with accum_out (TensorTensorReduce hangs this HW)
                        nc.vector.scalar_tensor_tensor(
                            scr[:], qt[:, HD * t:HD * (t + 1)], 1.0,
                            ps[:, 128 * i:128 * i + HD],
                            OP.mult, OP.mult, accum_out=scg[:, i:i + 1])
                    # probs cols for t=4g..4g+3 at stride SM
                    pr = probs[:].rearrange("p (t s) -> p t s", s=SM)[
                        :, 4 * g:4 * (g + 1), sm:sm + 1].squeeze(2)
                    nc.scalar.activation(pr, scg[:], AF.Exp, scale=SCALE)
                    for i in range(4):
                        t = 4 * g + i
                        vc = ps[:, 128 * i + HD:128 * (i + 1)]
                        pc = probs[:, t * SM + sm:t * SM + sm + 1]
                        at = acc[:, HD * t:HD * (t + 1)]
                        if sm == 0:
                            nc.vector.tensor_scalar_mul(at, vc, pc)
                        else:
                            nc.vector.scalar_tensor_tensor(
                                at, vc, pc, at, OP.mult, OP.add)

            # ---------------- normalize + back to channel-major ----------------
            osb = pp.tile([HD, P], f32, tag="osb")
            if mode == "skeleton":
                nc.vector.tensor_copy(osb[:], qsb[:])
            for t in (range(0) if mode == "skeleton" else range(32)):
                sume = sp.tile([128, 1], f32, tag="sume")
                nc.vector.tensor_reduce(
                    sume[:], probs[:, t * SM:(t + 1) * SM],
                    AX.X, OP.add)
                rec = sp.tile([128, 1], f32, tag="rec")
                nc.vector.reciprocal(rec[:], sume[:])
                at = acc[:, HD * t:HD * (t + 1)]
                nc.vector.tensor_scalar_mul(at, at, rec[:])
                ps = psp.tile([HD, 128], f32, tag="ps")
                nc.tensor.transpose(ps[:], at, ident[:])
                nc.scalar.copy(osb[:, 128 * t:128 * (t + 1)], ps[:])

            # ---------------- GroupNorm (one group = this head) ----------------
            st2 = sp.tile([HD, 2], f32, tag="st2")
            nc.vector.tensor_reduce(st2[:, 0:1], osb[:], AX.X, OP.add)
            osq = wp.tile([128, P], f32, tag="osq")
            nc.scalar.square(osq[0:HD, :], osb[:])
            nc.vector.tensor_reduce(st2[:, 1:2], osq[0:HD, :], AX.X, OP.add)
            psS = psp.tile([1, 2], f32, tag="ps")
            nc.tensor.matmul(psS[:], ones_col[:], st2[:], start=True, stop=True)
            mu2 = sp.tile([1, 2], f32, tag="mu2")
            nc.scalar.mul(mu2[:], psS[:], 1.0 / (HD * P))
            musq = sp.tile([1, 1], f32, tag="musq")
            nc.vector.tensor_tensor(musq[:], mu2[:, 0:1], mu2[:, 0:1], OP.mult)
            var = sp.tile([1, 1], f32, tag="var")
            nc.vector.scalar_tensor_tensor(
                var[:], musq[:], -1.0, mu2[:, 1:2], OP.mult, OP.add)
            eps_t = sp.tile([1, 1], f32, tag="eps")
            nc.gpsimd.memset(eps_t[:], GN_EPS)
            std = sp.tile([1, 1], f32, tag="std")
            nc.scalar.activation(std[:], var[:], AF.Sqrt, bias=eps_t[:])
            inv = sp.tile([1, 1], f32, tag="inv")
            nc.vector.reciprocal(inv[:], std[:])
            mi = sp.tile([1, 2], f32, tag="mi")
            nc.vector.tensor_copy(mi[:, 0:1], mu2[:, 0:1])
            nc.vector.tensor_copy(mi[:, 1:2], inv[:])
            psB = psp.tile([HD, 2], f32, tag="ps")
            nc.tensor.matmul(psB[:], ones_row[:], mi[:], start=True, stop=True)
            muinv = sp.tile([HD, 2], f32, tag="muinv")
            nc.scalar.copy(muinv[:], psB[:])
            s_ap = sp.tile([HD, 1], f32, tag="s_ap")
            nc.vector.tensor_tensor(s_ap[:], muinv[:, 1:2], gnsb_t[:, 0:1],
                                    OP.mult)
            tmp = sp.tile([HD, 1], f32, tag="tmpb")
            nc.vector.tensor_tensor(tmp[:], muinv[:, 0:1], s_ap[:], OP.mult)
            b_ap = sp.tile([HD, 1], f32, tag="b_ap")
            nc.vector.scalar_tensor_tensor(
                b_ap[:], tmp[:], -1.0, gnsb_t[:, 1:2], OP.mult, OP.add)
            nc.vector.tensor_scalar(osb[:], osb[:], s_ap[:], b_ap[:],
                                    OP.mult, OP.add)

            # ---------------- partial output projection ----------------
            for c in range(2):
                for j in range(8):
                    pso = psp.tile([128, 512], f32, tag="ps")
                    nc.tensor.matmul(
                        pso[:], wo_t[:, 128 * c:128 * (c + 1)],
                        osb[:, 512 * j:512 * (j + 1)], start=True, stop=True)
                    post = sp.tile([128, 512], bf16, tag="po_st")
                    nc.scalar.copy(post[:], pso[:])
                    nc.sync.dma_start(
                        po[128 * c:128 * (c + 1), 512 * j:512 * (j + 1)],
                        post[:])
    nc.compile()
    return nc


def _shift_tables(x, dir_W, dir_b):
    """Host-side: per-(b,h,sm) padded-corner offsets + blend weights."""
    avg = x.reshape(B, DIM, P).mean(axis=2)
    dirs = (avg @ dir_W.T + dir_b).reshape(B, HEADS, STRIPS, 2)
    dirs = dirs / np.maximum(np.linalg.norm(dirs, axis=-1, keepdims=True), 1e-6)
    t = np.linspace(-MAX_OFF, MAX_OFF, M)
    dx = (dirs[..., 0:1] * t * 0.5 * (W - 1)).reshape(B, HEADS, SM)
    dy = (dirs[..., 1:2] * t * 0.5 * (H - 1)).reshape(B, HEADS, SM)
    dxi = np.floor(dx)
    dyi = np.floor(dy)
    fx = (dx - dxi).astype(np.float32)
    fy = (dy - dyi).astype(np.float32)
    yo = (16 + dyi).astype(np.int32)              # [B,HEADS,SM]
    xo = (16 + dxi).astype(np.int32)
    w4 = np.stack([(1 - fy) * (1 - fx), (1 - fy) * fx,
                   fy * (1 - fx), fy * fx], axis=-1)   # [B,HEADS,SM,4]
    return yo, xo, w4.astype(np.float32)


def make_in_maps(inputs):
    x = np.asarray(inputs["x"], np.float32)
    Wq = np.asarray(inputs["Wq"], np.float32)
    Wk = np.asarray(inputs["Wk"], np.float32)
    Wv = np.asarray(inputs["Wv"], np.float32)
    Wo = np.asarray(inputs["Wo"], np.float32)
    dir_W = np.asarray(inputs["dir_W"], np.float32)
    dir_b = np.asarray(inputs["dir_b"], np.float32)
    gn_w = np.asarray(inputs["gn_w"], np.float32)
    gn_b = np.asarray(inputs["gn_b"], np.float32)
    yo, xo, w4 = _shift_tables(x, dir_W, dir_b)
    in_maps = []
    for core in range(N_CORES):
        b, h = core // HEADS, core % HEADS
        sl = slice(HD * h, HD * (h + 1))
        yoxo = np.concatenate([yo[b, h], yo[b, h] + 1,
                               xo[b, h], xo[b, h] + 1])[None, :]   # [1, 128]
        wts = w4[b, h].reshape(1, 4 * SM)
        gnsb = np.stack([gn_w[sl], gn_b[sl]], axis=1)              # [64, 2]
        in_maps.append({
            "xb": np.ascontiguousarray(x[b].reshape(DIM, P)).astype(bfloat16),
            "wq": np.ascontiguousarray(Wq[sl, :].T).astype(bfloat16),
            "wk": np.ascontiguousarray(Wk[sl, :].T).astype(bfloat16),
            "wv": np.ascontiguousarray(Wv[sl, :].T).astype(bfloat16),
            "wo": np.ascontiguousarray(Wo[:, sl].T),
            "yoxo": np.ascontiguousarray(yoxo),
            "wts": np.ascontiguousarray(wts),
            "gnsb": np.ascontiguousarray(gnsb),
        })
    return in_maps


def _make_launcher(nc):
    """Cached-jit SPMD launcher (run_bass_via_pjrt re-jits every call)."""
    import jax
    import concourse.mybir as mybir
    from concourse.bass2jax import (_bass_exec_p, partition_id_tensor,
                                    install_neuronx_cc_hook)
    from jax.sharding import Mesh, PartitionSpec
    from jax.experimental.shard_map import shard_map

    install_neuronx_cc_hook()
    pname = nc.partition_id_tensor.name if nc.partition_id_tensor else None
    in_names, out_names, out_avals, zero_shapes = [], [], [], []
    for alloc in nc.m.functions[0].allocations:
        if not isinstance(alloc, mybir.MemoryLocationSet):
            continue
        name = alloc.memorylocations[0].name
        if alloc.kind == "ExternalInput":
            if name != pname:
                in_names.append(name)
        elif alloc.kind == "ExternalOutput":
            out_names.append(name)
            shape = tuple(alloc.tensor_shape)
            dtype = mybir.dt.np(alloc.dtype)
            out_avals.append(jax.core.ShapedArray(shape, dtype))
            zero_shapes.append((shape, dtype))
    n_params, n_outs = len(in_names), len(out_avals)
    all_names = list(in_names) + list(out_names) + ([pname] if pname else [])
    donate = tuple(range(n_params, n_params + n_outs))

    def _body(*args):
        operands = list(args)
        if pname is not None:
            operands.append(partition_id_tensor())
        return tuple(_bass_exec_p.bind(
            *operands, out_avals=tuple(out_avals), in_names=tuple(all_names),
            out_names=tuple(out_names), lowering_input_output_aliases=(),
            sim_require_finite=True, sim_require_nnan=True, nc=nc))

    mesh = Mesh(np.asarray(jax.devices()[:N_CORES]), ("core",))
    sharded = jax.jit(
        shard_map(_body, mesh=mesh,
                  in_specs=(PartitionSpec("core"),) * (n_params + n_outs),
                  out_specs=(PartitionSpec("core"),) * n_outs,
                  check_rep=False),
        donate_argnums=donate, keep_unused=True)

    def launch(in_maps):
        concat_in = [
            np.concatenate([np.asarray(m[n]) for m in in_maps], axis=0)
            for n in in_names]
        concat_zeros = [np.zeros((N_CORES * s[0], *s[1:]), d)
                        for (s, d) in zero_shapes]
        outs = sharded(*concat_in, *concat_zeros)
        res = [np.asarray(o) for o in outs]
        return {name: res[i].reshape(N_CORES, *out_avals[i].shape)
                for i, name in enumerate(out_names)}
    return launch


def kernel(**inputs):
    x = np.asarray(inputs["x"], np.float32)
    bo = np.asarray(inputs["bo"], np.float32)
    if "launch" not in _CACHE:
        nc = _build_nc()
        _CACHE["launch"] = _make_launcher(nc)
    launch = _CACHE["launch"]
    in_maps = make_in_maps(inputs)
    res = launch(in_maps)
    po = res["po"].astype(np.float32)                 # [8, DIM, P]
    out = po.reshape(B, HEADS, DIM, P).sum(axis=1)
    out += bo[None, :, None] + x.reshape(B, DIM, P)
    return out.reshape(B, DIM, H, W).astype(np.float32)


# revision 4
# speedup vs baseline: 86.5565x; 1.1414x over previous
"""Full on-device Trainium2 kernel for nn_DeformableStripAttention.

Sharding: 8 cores = (b, h) pairs. Each core runs the whole per-head pipeline:
QKV projection, deformable strip sampling, attention, GroupNorm, and its
partial of the output projection. Host computes only the tiny shift tables
(from the direction head) and sums the 4 per-head partials per sample.

Key math: the sampling offset is constant across pixels, so grid_sample
reduces to a constant fractional shift of the whole image with replicate
border == 4 integer-shifted edge-clamped copies blended with fixed weights.
On device that is 4 fused multiply-adds reading a 96x96 replicate-padded
image at dynamic offsets (per strip-sample), with K and V packed on the
128 partitions.
"""
import sys
sys.path.insert(0, "/opt/trn_rl_repo")
import numpy as np
from ml_dtypes import bfloat16

DIM = 256
HEADS = 4
STRIPS = 4
M = 8
SM = STRIPS * M
MAX_OFF = 0.5
B = 2
H = 64
W = 64
HD = DIM // HEADS
P = H * W
PADW = 96
SCALE = HD ** -0.5
GN_EPS = 1e-5
N_CORES = 8

_CACHE = {}


def _build_nc(mode="full"):
    import concourse.bacc as bacc
    import concourse.mybir as mybir
    import concourse.tile as tile
    import concourse.bass as bass
    from concourse import masks

    f32 = mybir.dt.float32
    bf16 = mybir.dt.bfloat16
    i32 = mybir.dt.int32
    AF = mybir.ActivationFunctionType
    OP = mybir.AluOpType
    AX = mybir.AxisListType
    ds = bass.ds

    nc = bacc.Bacc("TRN2", target_bir_lowering=False, debug=False,
                   num_devices=N_CORES)

    xq = nc.dram_tensor("xq", [HD, P], bf16, kind="ExternalInput")
    xq_int = nc.dram_tensor("xq_int", [HD, P], bf16, kind="Internal")
    xg_int = nc.dram_tensor("xg_int", [DIM, P], bf16, kind="Internal")
    po_int = nc.dram_tensor("po_int", [DIM, P], f32, kind="Internal")
    rs_int = nc.dram_tensor("rs_int", [HD, P], f32, kind="Internal")
    GROUPS = [[0, 1, 2, 3], [4, 5, 6, 7]]
    wq = nc.dram_tensor("wq", [DIM, HD], bf16, kind="ExternalInput")
    wk = nc.dram_tensor("wk", [DIM, HD], bf16, kind="ExternalInput")
    wv = nc.dram_tensor("wv", [DIM, HD], bf16, kind="ExternalInput")
    wo = nc.dram_tensor("wo", [HD, DIM], f32, kind="ExternalInput")
    yoxo = nc.dram_tensor("yoxo", [1, 4 * SM], i32, kind="ExternalInput")
    wts = nc.dram_tensor("wts", [1, 4 * SM], f32, kind="ExternalInput")
    gnsb = nc.dram_tensor("gnsb", [HD, 2], f32, kind="ExternalInput")
    pos = nc.dram_tensor("pos", [HD, P], bf16, kind="ExternalOutput")

    with tile.TileContext(nc) as tc:
        with tc.tile_pool(name="persist", bufs=1) as pp, \
             tc.tile_pool(name="work", bufs=2) as wp, \
             tc.tile_pool(name="small", bufs=3) as sp, \
             tc.tile_pool(name="psum", bufs=5, space="PSUM") as psp:

            # ---------------- constants & inputs to SBUF ----------------
            ident = pp.tile([128, 128], f32, tag="ident")
            masks.make_identity(nc, ident[:])
            identb = pp.tile([128, 128], bf16, tag="identb")
            masks.make_identity(nc, identb[:])
            ones_col = pp.tile([HD, 1], f32, tag="onesc")
            nc.gpsimd.memset(ones_col[:], 1.0)
            ones_row = pp.tile([1, HD], f32, tag="onesr")
            nc.gpsimd.memset(ones_row[:], 1.0)

            nc.sync.dma_start(xq_int[:], xq[:])
            nc.gpsimd.collective_compute(
                "AllGather", OP.bypass, GROUPS,
                ins=[xq_int[:]], outs=[xg_int[:]])
            xt = [pp.tile([128, P], bf16, tag=f"xt{i}", name=f"xt{i}")
                  for i in range(2)]
            for c in range(2):
                nc.sync.dma_start(xt[c][:], xg_int[128 * c:128 * (c + 1), :])
            wq_t = pp.tile([128, 2 * HD], bf16, tag="wq")
            wk_t = pp.tile([128, 2 * HD], bf16, tag="wk")
            wv_t = pp.tile([128, 2 * HD], bf16, tag="wv")
            wo_t = pp.tile([HD, DIM], f32, tag="wo")
            for t_, d_ in ((wq_t, wq), (wk_t, wk), (wv_t, wv)):
                for c in range(2):
                    nc.sync.dma_start(t_[:, HD * c:HD * (c + 1)],
                                      d_[128 * c:128 * (c + 1), :])
            nc.sync.dma_start(wo_t[:], wo[:])
            yoxo_t = pp.tile([1, 4 * SM], i32, tag="yoxo")
            nc.sync.dma_start(yoxo_t[:], yoxo[:])
            wts_row = pp.tile([1, 4 * SM], f32, tag="wtsr")
            nc.sync.dma_start(wts_row[:], wts[:])
            ones_r128 = pp.tile([1, 128], f32, tag="onesr128")
            nc.gpsimd.memset(ones_r128[:], 1.0)
            wts_t = pp.tile([128, 4 * SM], f32, tag="wts")
            psw = psp.tile([128, 4 * SM], f32, tag="ps")
            nc.tensor.matmul(psw[:], ones_r128[:], wts_row[:],
                             start=True, stop=True)
            nc.scalar.copy(wts_t[:], psw[:])
            gnsb_t = pp.tile([HD, 2], f32, tag="gnsb")
            nc.sync.dma_start(gnsb_t[:], gnsb[:])

            # ---------------- QKV projections ----------------
            # K/V go straight into the interior of the padded KV image.
            kvpad = pp.tile([128, PADW * PADW], bf16, tag="kvpad")
            kvp3 = kvpad[:].rearrange("p (r c) -> p r c", c=PADW)
            qsb = pp.tile([HD, P], bf16, tag="qsb")

            for wt, kind in ((wq_t, "q"), (wk_t, "k"), (wv_t, "v")):
                for j in range(8):
                    ps = psp.tile([HD, 512], f32, tag="ps")
                    for c in range(2):
                        nc.tensor.matmul(
                            ps[:], wt[:, HD * c:HD * (c + 1)],
                            xt[c][:, 512 * j:512 * (j + 1)],
                            start=(c == 0), stop=(c == 1))
                    if kind == "q":
                        nc.scalar.copy(qsb[:, 512 * j:512 * (j + 1)], ps[:])
                    else:
                        prow = 0 if kind == "k" else HD
                        dst = kvp3[prow:prow + HD,
                                   16 + 8 * j:16 + 8 * (j + 1), 16:16 + W]
                        nc.scalar.copy(dst, ps[:].rearrange(
                            "p (r c) -> p r c", c=W))

            # ---------------- replicate-pad borders ----------------
            nc.vector.tensor_copy(
                kvp3[:, 16:16 + H, 0:16],
                kvp3[:, 16:16 + H, 16:17].to_broadcast([128, H, 16]))
            nc.vector.tensor_copy(
                kvp3[:, 16:16 + H, 16 + W:PADW],
                kvp3[:, 16:16 + H, 15 + W:16 + W].to_broadcast([128, H, 16]))
            nc.vector.tensor_copy(
                kvp3[:, 0:16, :],
                kvp3[:, 16:17, :].to_broadcast([128, 16, PADW]))
            nc.vector.tensor_copy(
                kvp3[:, 16 + H:PADW, :],
                kvp3[:, 15 + H:16 + H, :].to_broadcast([128, 16, PADW]))

            # ---------------- Q transposed to pixel-major ----------------
            qt = pp.tile([128, 2048], bf16, tag="qt")      # [p, t*64+ch]
            for t in range(32):
                ps = psp.tile([128, HD], bf16, tag="ps")
                nc.tensor.transpose(ps[:], qsb[:, 128 * t:128 * (t + 1)],
                                    identb[0:HD, 0:HD])
                nc.scalar.copy(qt[:, HD * t:HD * (t + 1)], ps[:])

            # ---------------- main sampling + attention loop ----------------
            acc = pp.tile([128, 2048], f32, tag="acc")     # [p, t*64+ch]
            probs = pp.tile([128, SM * 32], f32, tag="probs")  # col = t*SM+sm

            # Two y- and two x-corner registers, reloaded each sm. No symbolic
            # arithmetic on them (host sends all four coords) so no stale
            # value-cache entries can survive a reload.
            cregs = [nc.vector.alloc_register(f"creg{i}") for i in range(4)]
            cvals = [bass.RuntimeValue(r, min_val=0, max_val=32) for r in cregs]

            sm_range = range(0) if mode == "skeleton" else range(SM)
            for sm in sm_range:
                if mode == "full":
                    for i in range(4):
                        nc.vector.reg_load(
                            cregs[i], yoxo_t[0:1, SM * i + sm:SM * i + sm + 1])
                skv = wp.tile([128, P], bf16, tag="skv")
                skv3 = skv[:].rearrange("p (a b) -> p a b", b=W)
                for ci, (cy, cx) in enumerate(((0, 0), (0, 1), (1, 0), (1, 1))):
                    if mode == "full":
                        src = kvp3[:, ds(cvals[cy], H), ds(cvals[2 + cx], W)]
                    else:
                        src = kvp3[:, 16 + cy:16 + cy + H, 16 + cx:16 + cx + W]
                    wcol = wts_t[:, 4 * sm + ci:4 * sm + ci + 1]
                    if ci == 0:
                        nc.vector.tensor_scalar_mul(skv3, src, wcol)
                    else:
                        nc.vector.scalar_tensor_tensor(
                            skv3, src, wcol, skv3, OP.mult, OP.add)

                for g in range(8):
                    ps = psp.tile([128, 512], bf16, tag="ps")
                    for i in range(4):
                        t = 4 * g + i
                        nc.tensor.transpose(
                            ps[:, 128 * i:128 * (i + 1)],
                            skv[:, 128 * t:128 * (t + 1)], identb[:])
                    scg = sp.tile([128, 4], f32, tag="scg")
                    for i in range(4):
                        t = 4 * g + i
                        scr = sp.tile([128, HD], bf16, tag="ttr_scr")
                        # dot(qt_t, sampled K) per pixel: elementwise product
                        # with accum_out (TensorTensorReduce hangs this HW)
                        nc.vector.scalar_tensor_tensor(
                            scr[:], qt[:, HD * t:HD * (t + 1)], 1.0,
                            ps[:, 128 * i:128 * i + HD],
                            OP.mult, OP.mult, accum_out=scg[:, i:i + 1])
                    # probs cols for t=4g..4g+3 at stride SM
                    pr = probs[:].rearrange("p (t s) -> p t s", s=SM)[
                        :, 4 * g:4 * (g + 1), sm:sm + 1].squeeze(2)
                    nc.scalar.activation(pr, scg[:], AF.Exp, scale=SCALE)
                    for i in range(4):
                        t = 4 * g + i
                        vc = ps[:, 128 * i + HD:128 * (i + 1)]
                        pc = probs[:, t * SM + sm:t * SM + sm + 1]
                        at = acc[:, HD * t:HD * (t + 1)]
                        if sm == 0:
                            nc.vector.tensor_scalar_mul(at, vc, pc)
                        else:
                            nc.vector.scalar_tensor_tensor(
                                at, vc, pc, at, OP.mult, OP.add)

            # ---------------- normalize + back to channel-major ----------------
            osb = pp.tile([HD, P], f32, tag="osb")
            if mode == "skeleton":
                nc.vector.tensor_copy(osb[:], qsb[:])
            for t in (range(0) if mode == "skeleton" else range(32)):
                sume = sp.tile([128, 1], f32, tag="sume")
                nc.vector.tensor_reduce(
                    sume[:], probs[:, t * SM:(t + 1) * SM],
                    AX.X, OP.add)
                rec = sp.tile([128, 1], f32, tag="rec")
                nc.vector.reciprocal(rec[:], sume[:])
                at = acc[:, HD * t:HD * (t + 1)]
                nc.vector.tensor_scalar_mul(at, at, rec[:])
                ps = psp.tile([HD, 128], f32, tag="ps")
                nc.tensor.transpose(ps[:], at, ident[:])
                nc.scalar.copy(osb[:, 128 * t:128 * (t + 1)], ps[:])

            # ---------------- GroupNorm (one group = this head) ----------------
            st2 = sp.tile([HD, 2], f32, tag="st2")
            nc.vector.tensor_reduce(st2[:, 0:1], osb[:], AX.X, OP.add)
            osq = wp.tile([128, P], f32, tag="osq")
            nc.scalar.square(osq[0:HD, :], osb[:])
            nc.vector.tensor_reduce(st2[:, 1:2], osq[0:HD, :], AX.X, OP.add)
            psS = psp.tile([1, 2], f32, tag="ps")
            nc.tensor.matmul(psS[:], ones_col[:], st2[:], start=True, stop=True)
            mu2 = sp.tile([1, 2], f32, tag="mu2")
            nc.scalar.mul(mu2[:], psS[:], 1.0 / (HD * P))
            musq = sp.tile([1, 1], f32, tag="musq")
            nc.vector.tensor_tensor(musq[:], mu2[:, 0:1], mu2[:, 0:1], OP.mult)
            var = sp.tile([1, 1], f32, tag="var")
            nc.vector.scalar_tensor_tensor(
                var[:], musq[:], -1.0, mu2[:, 1:2], OP.mult, OP.add)
            eps_t = sp.tile([1, 1], f32, tag="eps")
            nc.gpsimd.memset(eps_t[:], GN_EPS)
            std = sp.tile([1, 1], f32, tag="std")
            nc.scalar.activation(std[:], var[:], AF.Sqrt, bias=eps_t[:])
            inv = sp.tile([1, 1], f32, tag="inv")
            nc.vector.reciprocal(inv[:], std[:])
            mi = sp.tile([1, 2], f32, tag="mi")
            nc.vector.tensor_copy(mi[:, 0:1], mu2[:, 0:1])
            nc.vector.tensor_copy(mi[:, 1:2], inv[:])
            psB = psp.tile([HD, 2], f32, tag="ps")
            nc.tensor.matmul(psB[:], ones_row[:], mi[:], start=True, stop=True)
            muinv = sp.tile([HD, 2], f32, tag="muinv")
            nc.scalar.copy(muinv[:], psB[:])
            s_ap = sp.tile([HD, 1], f32, tag="s_ap")
            nc.vector.tensor_tensor(s_ap[:], muinv[:, 1:2], gnsb_t[:, 0:1],
                                    OP.mult)
            tmp = sp.tile([HD, 1], f32, tag="tmpb")
            nc.vector.tensor_tensor(tmp[:], muinv[:, 0:1], s_ap[:], OP.mult)
            b_ap = sp.tile([HD, 1], f32, tag="b_ap")
            nc.vector.scalar_tensor_tensor(
                b_ap[:], tmp[:], -1.0, gnsb_t[:, 1:2], OP.mult, OP.add)
            nc.vector.tensor_scalar(osb[:], osb[:], s_ap[:], b_ap[:],
                                    OP.mult, OP.add)

            # ---------------- partial output projection ----------------
            for c in range(2):
                for j in range(8):
                    pso = psp.tile([128, 512], f32, tag="ps")
                    nc.tensor.matmul(
                        pso[:], wo_t[:, 128 * c:128 * (c + 1)],
                        osb[:, 512 * j:512 * (j + 1)], start=True, stop=True)
                    post = sp.tile([128, 512], f32, tag="po_st")
                    nc.scalar.copy(post[:], pso[:])
                    nc.sync.dma_start(
                        po_int[128 * c:128 * (c + 1), 512 * j:512 * (j + 1)],
                        post[:])
            nc.gpsimd.collective_compute(
                "ReduceScatter", OP.add, GROUPS,
                ins=[po_int[:]], outs=[rs_int[:]])
            rs_sb = pp.tile([HD, P], f32, tag="rs_sb")
            nc.sync.dma_start(rs_sb[:], rs_int[:])
            rs_bf = pp.tile([HD, P], bf16, tag="rs_bf")
            nc.vector.tensor_copy(rs_bf[:], rs_sb[:])
            nc.sync.dma_start(pos[:], rs_bf[:])
    nc.compile()
    return nc


def _shift_tables(x, dir_W, dir_b):
    """Host-side: per-(b,h,sm) padded-corner offsets + blend weights."""
    avg = x.reshape(B, DIM, P).mean(axis=2)
    dirs = (avg @ dir_W.T + dir_b).reshape(B, HEADS, STRIPS, 2)
    dirs = dirs / np.maximum(np.linalg.norm(dirs, axis=-1, keepdims=True), 1e-6)
    t = np.linspace(-MAX_OFF, MAX_OFF, M)
    dx = (dirs[..., 0:1] * t * 0.5 * (W - 1)).reshape(B, HEADS, SM)
    dy = (dirs[..., 1:2] * t * 0.5 * (H - 1)).reshape(B, HEADS, SM)
    dxi = np.floor(dx)
    dyi = np.floor(dy)
    fx = (dx - dxi).astype(np.float32)
    fy = (dy - dyi).astype(np.float32)
    yo = (16 + dyi).astype(np.int32)              # [B,HEADS,SM]
    xo = (16 + dxi).astype(np.int32)
    w4 = np.stack([(1 - fy) * (1 - fx), (1 - fy) * fx,
                   fy * (1 - fx), fy * fx], axis=-1)   # [B,HEADS,SM,4]
    return yo, xo, w4.astype(np.float32)


def make_in_maps(inputs):
    x = np.asarray(inputs["x"], np.float32)
    Wq = np.asarray(inputs["Wq"], np.float32)
    Wk = np.asarray(inputs["Wk"], np.float32)
    Wv = np.asarray(inputs["Wv"], np.float32)
    Wo = np.asarray(inputs["Wo"], np.float32)
    dir_W = np.asarray(inputs["dir_W"], np.float32)
    dir_b = np.asarray(inputs["dir_b"], np.float32)
    gn_w = np.asarray(inputs["gn_w"], np.float32)
    gn_b = np.asarray(inputs["gn_b"], np.float32)
    yo, xo, w4 = _shift_tables(x, dir_W, dir_b)
    in_maps = []
    for core in range(N_CORES):
        b, h = core // HEADS, core % HEADS
        sl = slice(HD * h, HD * (h + 1))
        yoxo = np.concatenate([yo[b, h], yo[b, h] + 1,
                               xo[b, h], xo[b, h] + 1])[None, :]   # [1, 128]
        wts = w4[b, h].reshape(1, 4 * SM)
        gnsb = np.stack([gn_w[sl], gn_b[sl]], axis=1)              # [64, 2]
        in_maps.append({
            "xq": np.ascontiguousarray(x[b].reshape(DIM, P)[sl]).astype(bfloat16),
            "wq": np.ascontiguousarray(Wq[sl, :].T).astype(bfloat16),
            "wk": np.ascontiguousarray(Wk[sl, :].T).astype(bfloat16),
            "wv": np.ascontiguousarray(Wv[sl, :].T).astype(bfloat16),
            "wo": np.ascontiguousarray(Wo[:, sl].T),
            "yoxo": np.ascontiguousarray(yoxo),
            "wts": np.ascontiguousarray(wts),
            "gnsb": np.ascontiguousarray(gnsb),
        })
    return in_maps


def _make_launcher(nc):
    """Cached-jit SPMD launcher (run_bass_via_pjrt re-jits every call)."""
    import jax
    import concourse.mybir as mybir
    from concourse.bass2jax import (_bass_exec_p, partition_id_tensor,
                                    install_neuronx_cc_hook)
    from jax.sharding import Mesh, PartitionSpec
    from jax.experimental.shard_map import shard_map

    install_neuronx_cc_hook()
    pname = nc.partition_id_tensor.name if nc.partition_id_tensor else None
    in_names, out_names, out_avals, zero_shapes = [], [], [], []
    for alloc in nc.m.functions[0].allocations:
        if not isinstance(alloc, mybir.MemoryLocationSet):
            continue
        name = alloc.memorylocations[0].name
        if alloc.kind == "ExternalInput":
            if name != pname:
                in_names.append(name)
        elif alloc.kind == "ExternalOutput":
            out_names.append(name)
            shape = tuple(alloc.tensor_shape)
            dtype = mybir.dt.np(alloc.dtype)
            out_avals.append(jax.core.ShapedArray(shape, dtype))
            zero_shapes.append((shape, dtype))
    n_params, n_outs = len(in_names), len(out_avals)
    all_names = list(in_names) + list(out_names) + ([pname] if pname else [])
    donate = tuple(range(n_params, n_params + n_outs))

    def _body(*args):
        operands = list(args)
        if pname is not None:
            operands.append(partition_id_tensor())
        return tuple(_bass_exec_p.bind(
            *operands, out_avals=tuple(out_avals), in_names=tuple(all_names),
            out_names=tuple(out_names), lowering_input_output_aliases=(),
            sim_require_finite=True, sim_require_nnan=True, nc=nc))

    mesh = Mesh(np.asarray(jax.devices()[:N_CORES]), ("core",))
    sharded = jax.jit(
        shard_map(_body, mesh=mesh,
                  in_specs=(PartitionSpec("core"),) * (n_params + n_outs),
                  out_specs=(PartitionSpec("core"),) * n_outs,
                  check_rep=False),
        donate_argnums=donate, keep_unused=True)

    def launch(in_maps):
        concat_in = [
            np.concatenate([np.asarray(m[n]) for m in in_maps], axis=0)
            for n in in_names]
        concat_zeros = [np.zeros((N_CORES * s[0], *s[1:]), d)
                        for (s, d) in zero_shapes]
        outs = sharded(*concat_in, *concat_zeros)
        res = [np.asarray(o) for o in outs]
        return {name: res[i].reshape(N_CORES, *out_avals[i].shape)
                for i, name in enumerate(out_names)}
    return launch


def kernel(**inputs):
    x = np.asarray(inputs["x"], np.float32)
    bo = np.asarray(inputs["bo"], np.float32)
    if "launch" not in _CACHE:
        nc = _build_nc()
        _CACHE["launch"] = _make_launcher(nc)
    launch = _CACHE["launch"]
    in_maps = make_in_maps(inputs)
    res = launch(in_maps)
    pos = res["pos"].astype(np.float32)               # [8, HD, P]
    out = pos.reshape(B, DIM, P)
    out += bo[None, :, None] + x.reshape(B, DIM, P)
    return out.reshape(B, DIM, H, W).astype(np.float32)
